# revision 7
# baseline (speedup 1.0000x reference)
# GQA kernel for trn2, 8 cores.
#
# Sharding: core = (b, g) for b in {0,1} batches x g in {0..3} kv-heads.
# Each core computes its batch's 4 q-heads belonging to kv-head g:
#   Q^T = (Wq_g/8) @ x_q^T          [256, S]   (1/sqrt(dk) folded into Wq)
#   K^T = Wk_g @ x_k^T              [64, S]
#   V~  = (x_v @ Wv_g^T | 1)        [S, 65]    (ones column -> softmax sums)
#   scoresT[k,q] = K^T.T-free matmul; exp on ACT; ctxU^T = V~^T-style matmul
#   row 64 of ctxU = softmax denominators; broadcast 1/sums via PE outer
#   product; normalize attnT and ctx^T on DVE; out_part = ctx^T-proj with
#   Wo[:, g-cols]; host sums the 4 partials per batch (the Wo all-reduce).
#
# attn is produced TRANSPOSED ([k,q]) on device and returned as a numpy
# transpose view -- no device-side transpose of the big matrix is needed.
#
# All matmuls run as float32r (fp32 bits, fast PE mode, 1 cycle/row at
# moving>=256). Biases bq/bk/bv are applied on-device (free, folded into the
# PSUM->SBUF copies); bo is added on host after the partial-sum reduce.

import numpy as np

import concourse.bass as bass
from concourse import bacc
import concourse.mybir as mybir
import concourse.tile as tile
from concourse.bass_utils import run_bass_kernel_spmd
from concourse.masks import make_identity

B, DM = 2, 1024
H, KVH, DK = 16, 4, 64
REP = H // KVH          # 4 q-heads per core
QH = REP * DK           # 256
F32 = mybir.dt.float32
F32R = mybir.dt.float32r
ALU_MULT = mybir.AluOpType.mult
EXP = mybir.ActivationFunctionType.Exp
IDENT = mybir.ActivationFunctionType.Identity


def r(ap):
    return ap if ap.dtype == F32R else ap.bitcast(F32R)


def build(S=2048):
    """Build the per-core Bass module (SPMD; same program all 8 cores)."""
    NQB = S // 512       # q blocks of 512
    NKC = S // 128       # k chunks of 128
    CCH = DM // 128      # contraction chunks for projections
    nc = bacc.Bacc("TRN2", target_bir_lowering=False, debug=False)

    xq = nc.declare_dram_parameter("xq_T", [DM, S], F32R, isOutput=False)
    xk = nc.declare_dram_parameter("xk_T", [DM, S], F32R, isOutput=False)
    xv = nc.declare_dram_parameter("xv_T", [DM, S], F32R, isOutput=False)
    wq = nc.declare_dram_parameter("wqT", [DM, QH], F32R, isOutput=False)
    wk = nc.declare_dram_parameter("wkT", [DM, DK], F32R, isOutput=False)
    wv = nc.declare_dram_parameter("wvT", [DM, DK], F32R, isOutput=False)
    wo = nc.declare_dram_parameter("woT", [QH, DM], F32R, isOutput=False)
    bq = nc.declare_dram_parameter("bq2", [64, REP], F32, isOutput=False)
    bk = nc.declare_dram_parameter("bk2", [64, 1], F32, isOutput=False)
    bv = nc.declare_dram_parameter("bv2", [64, 1], F32, isOutput=False)
    attnT = nc.declare_dram_parameter("attnT", [REP, S, S], F32, isOutput=True)
    outp = nc.declare_dram_parameter("outp", [S, DM], F32, isOutput=True)

    with tile.TileContext(nc) as tc:
        with tc.tile_pool(name="persist", bufs=1) as pp:
            qT_sb = pp.tile([64, REP, S], F32R)     # Q^T per head, base partition 0
            kT_sb = pp.tile([64, S], F32R)
            v_sb = pp.tile([128, NKC, 65], F32R)    # V~[k,d|1] per k-chunk
            ctx_sb = pp.tile([64, REP, S], F32R)    # ctx^T per head, base partition 0
            wo_sb = pp.tile([64, REP, DM], F32R)
            ones_sb = pp.tile([65, 128], F32R)  # row 64 used (matches sums base partition)
            ident = pp.tile([128, 128], F32)
            bq_sb = pp.tile([64, REP], F32)
            bk_sb = pp.tile([64, 1], F32)
            bv_sb = pp.tile([64, 1], F32)

            ones_f = pp.tile([65, 128], F32)
            nc.gpsimd.memset(ones_f[:], 1.0)
            nc.scalar.copy(ones_sb[:], ones_f[:])
            make_identity(nc, ident[:])
            nc.sync.dma_start(out=bq_sb[:], in_=bq[:])
            nc.sync.dma_start(out=bk_sb[:], in_=bk[:])
            nc.sync.dma_start(out=bv_sb[:], in_=bv[:])
            for hh in range(REP):
                nc.sync.dma_start(out=wo_sb[:, hh, :], in_=wo[hh * 64:(hh + 1) * 64, :])

            # ---------------- Phase A: projections ----------------
            with (
                tc.tile_pool(name="xload", bufs=8) as xpool,
                tc.tile_pool(name="wload", bufs=1) as wpool,
                tc.tile_pool(name="prps", bufs=3, space="PSUM") as prps,
                tc.tile_pool(name="vtps", bufs=2, space="PSUM") as vtps,
            ):
                wq_sb = wpool.tile([128, CCH, QH], F32R)
                wk_sb = wpool.tile([128, CCH, DK], F32R)
                wv_sb = wpool.tile([128, CCH, DK], F32R)
                for cc in range(CCH):
                    nc.sync.dma_start(out=wq_sb[:, cc, :], in_=wq[cc * 128:(cc + 1) * 128, :])
                    nc.sync.dma_start(out=wk_sb[:, cc, :], in_=wk[cc * 128:(cc + 1) * 128, :])
                    nc.sync.dma_start(out=wv_sb[:, cc, :], in_=wv[cc * 128:(cc + 1) * 128, :])

                NPH = S // 1024 if S >= 1024 else 1
                PHW = min(S, 1024)  # psum tile width
                NMV = PHW // 512 if PHW >= 512 else 1
                MVW = min(PHW, 512)

                def load_x(x_dram, tag):
                    tiles = []
                    for cc in range(CCH):
                        t = xpool.tile([128, S], F32R, name=f"x_{tag}_{cc}", tag="xt")
                        nc.sync.dma_start(out=t[:], in_=x_dram[cc * 128:(cc + 1) * 128, :])
                        tiles.append(t)
                    return tiles

                # Q projection (per head so every head sits at base partition 0)
                xt = load_x(xq, "q")
                for h in range(REP):
                    for ph in range(NPH):
                        pq = prps.tile([64, PHW], F32, name="pq", tag="prj")
                        for cc in range(CCH):
                            for mv in range(NMV):
                                nc.tensor.matmul(
                                    pq[:, mv * MVW:(mv + 1) * MVW],
                                    r(wq_sb[:, cc, h * 64:(h + 1) * 64]),
                                    r(xt[cc][:, ph * PHW + mv * MVW: ph * PHW + mv * MVW + MVW]),
                                    start=(cc == 0), stop=(cc == CCH - 1),
                                )
                        nc.scalar.activation(
                            qT_sb[:, h, ph * PHW:(ph + 1) * PHW], pq[:],
                            IDENT, bias=bq_sb[:, h:h + 1])

                # K projection
                xt = load_x(xk, "k")
                for ph in range(NPH):
                    pk = prps.tile([64, PHW], F32, name="pk", tag="prj")
                    for cc in range(CCH):
                        for mv in range(NMV):
                            nc.tensor.matmul(
                                pk[:, mv * MVW:(mv + 1) * MVW],
                                r(wk_sb[:, cc, :]),
                                r(xt[cc][:, ph * PHW + mv * MVW: ph * PHW + mv * MVW + MVW]),
                                start=(cc == 0), stop=(cc == CCH - 1),
                            )
                    nc.scalar.activation(
                        kT_sb[:, ph * PHW:(ph + 1) * PHW], pk[:],
                        IDENT, bias=bk_sb[:])

                # V projection -> vT (transient) -> transpose into v_sb
                vT_sb = wpool.tile([64, S], F32)
                xt = load_x(xv, "v")
                for ph in range(NPH):
                    pv = prps.tile([64, PHW], F32, name="pv", tag="prj")
                    for cc in range(CCH):
                        for mv in range(NMV):
                            nc.tensor.matmul(
                                pv[:, mv * MVW:(mv + 1) * MVW],
                                r(wv_sb[:, cc, :]),
                                r(xt[cc][:, ph * PHW + mv * MVW: ph * PHW + mv * MVW + MVW]),
                                start=(cc == 0), stop=(cc == CCH - 1),
                            )
                    nc.scalar.activation(
                        vT_sb[:, ph * PHW:(ph + 1) * PHW], pv[:],
                        IDENT, bias=bv_sb[:])
                for kc in range(NKC):
                    vt = vtps.tile([128, 64], F32, name="vt")
                    nc.tensor.transpose(vt[:], vT_sb[:, kc * 128:(kc + 1) * 128],
                                        ident[0:64, 0:64])
                    nc.vector.tensor_copy(v_sb[:, kc, 0:64], vt[:])
                onc_f = wpool.tile([128, NKC], F32)
                nc.gpsimd.memset(onc_f[:], 1.0)
                nc.scalar.copy(v_sb[:, :, 64], onc_f[:])

            # ---------------- Phase B: attention ----------------
            with (
                tc.tile_pool(name="scps", bufs=2, space="PSUM") as scps,
                tc.tile_pool(name="cups", bufs=2, space="PSUM") as cups,
                tc.tile_pool(name="bcps", bufs=2, space="PSUM") as bcps,
                tc.tile_pool(name="expp", bufs=2) as expp,
                tc.tile_pool(name="attp", bufs=6) as attp,
                tc.tile_pool(name="smal", bufs=4) as smal,
            ):
                for h in range(REP):
                    for qb in range(NQB):
                        q0 = qb * 512
                        exp_sb = expp.tile([128, NKC, 512], F32R, name="exp_sb")
                        cu = cups.tile([65, 512], F32, name="cu")
                        for kcg in range(NKC // 2):
                            sc = scps.tile([128, 2, 512], F32, name="sc")
                            for j in range(2):
                                kc = kcg * 2 + j
                                nc.tensor.matmul(
                                    sc[:, j, :],
                                    r(kT_sb[:, kc * 128:(kc + 1) * 128]),
                                    r(qT_sb[:, h, q0:q0 + 512]),
                                    start=True, stop=True,
                                )
                            nc.scalar.activation(
                                exp_sb[:, kcg * 2:kcg * 2 + 2, :], sc[:], EXP)
                            for j in range(2):
                                kc = kcg * 2 + j
                                nc.tensor.matmul(
                                    cu[:],
                                    r(v_sb[:, kc, :]),
                                    r(exp_sb[:, kc, :]),
                                    start=(kc == 0), stop=(kc == NKC - 1),
                                )
                        inv_sb = smal.tile([65, 512], F32R, name="inv_sb")
                        with nc.allow_low_precision(reason="f32r is 4-byte fp32 bits"):
                            nc.vector.reciprocal(inv_sb[64:65, :], cu[64:65, :])
                        bc = bcps.tile([128, 512], F32, name="bc")
                        nc.tensor.matmul(bc[:], r(ones_sb[64:65, :]), r(inv_sb[64:65, :]),
                                         start=True, stop=True)
                        bc_sb = smal.tile([128, 512], F32, name="bc_sb")
                        nc.vector.tensor_copy(bc_sb[:], bc[:])
                        # normalized ctx^T slice for this head/q-block
                        nc.vector.tensor_tensor(
                            ctx_sb[:, h, q0:q0 + 512],
                            cu[0:64, :], bc_sb[0:64, :], ALU_MULT)
                        # normalized attnT tiles -> DRAM
                        for kc in range(NKC):
                            at = attp.tile([128, 512], F32, name="at")
                            nc.vector.tensor_tensor(
                                at[:], exp_sb[:, kc, :], bc_sb[:], ALU_MULT)
                            nc.gpsimd.dma_start(
                                out=attnT[h, kc * 128:(kc + 1) * 128, q0:q0 + 512],
                                in_=at[:])

            # ---------------- Phase C: output projection ----------------
            with (
                tc.tile_pool(name="pops", bufs=4, space="PSUM") as pops,
                tc.tile_pool(name="posb", bufs=4) as posb,
            ):
                for qt in range(S // 128):
                    for nb in range(2):
                        po = pops.tile([128, 512], F32, name="po")
                        for hh in range(REP):
                            nc.tensor.matmul(
                                po[:],
                                r(ctx_sb[:, hh, qt * 128:(qt + 1) * 128]),
                                r(wo_sb[:, hh, nb * 512:(nb + 1) * 512]),
                                start=(hh == 0), stop=(hh == REP - 1),
                            )
                        out_sb = posb.tile([128, 512], F32, name="out_sb")
                        nc.scalar.copy(out_sb[:], po[:])
                        nc.gpsimd.dma_start(
                            out=outp[qt * 128:(qt + 1) * 128, nb * 512:(nb + 1) * 512],
                            in_=out_sb[:])
    nc.compile()
    return nc


def make_core_inputs(query, key, value, Wq, bq, Wk, bk, Wv, bv, Wo, S):
    """Host-side shard prep: per-core input dicts (core = b*4 + g)."""
    scale = 1.0 / np.sqrt(DK)
    in_maps = []
    xT = {}
    for b in range(B):
        xT[b] = tuple(np.ascontiguousarray(a[b].T) for a in (query, key, value))
    for b in range(B):
        for g in range(KVH):
            qs, ks = slice(g * QH, (g + 1) * QH), slice(g * DK, (g + 1) * DK)
            in_maps.append({
                "xq_T": xT[b][0], "xk_T": xT[b][1], "xv_T": xT[b][2],
                "wqT": np.ascontiguousarray((Wq[qs] * scale).T),
                "wkT": np.ascontiguousarray(Wk[ks].T),
                "wvT": np.ascontiguousarray(Wv[ks].T),
                "woT": np.ascontiguousarray(Wo[:, qs].T),
                "bq2": np.ascontiguousarray((bq[qs] * scale).reshape(REP, 64).T),
                "bk2": np.ascontiguousarray(bk[ks].reshape(64, 1)),
                "bv2": np.ascontiguousarray(bv[ks].reshape(64, 1)),
            })
    return in_maps


def assemble(results, bo, S):
    """Host-side gather: sum Wo partials per batch, stitch attn (as a view)."""
    out = np.empty((B, S, DM), np.float32)
    attn_kq = np.empty((B, H, S, S), np.float32)
    for b in range(B):
        acc = None
        for g in range(KVH):
            res = results[b * KVH + g]
            acc = res["outp"] if acc is None else acc + res["outp"]
            attn_kq[b, g * REP:(g + 1) * REP] = res["attnT"]
        out[b] = acc + np.asarray(bo, np.float32)[None, :]
    return out, attn_kq.transpose(0, 1, 3, 2)


def kernel(query, key, value, Wq, bq, Wk, bk, Wv, bv, Wo, bo):
    S = query.shape[1]
    nc = build(S)
    in_maps = make_core_inputs(
        np.asarray(query, np.float32), np.asarray(key, np.float32),
        np.asarray(value, np.float32), np.asarray(Wq, np.float32),
        np.asarray(bq, np.float32), np.asarray(Wk, np.float32),
        np.asarray(bk, np.float32), np.asarray(Wv, np.float32),
        np.asarray(bv, np.float32), np.asarray(Wo, np.float32), S)
    res = run_bass_kernel_spmd(nc, in_maps, list(range(8))).results
    return assemble(res, bo, S)
